# revision 30
# baseline (speedup 1.0000x reference)
"""GCNConv (graph message passing) on 8 Trainium2 NeuronCores — Bass/Tile.

out = a + (a @ Wres + bres),  a = relu(segment_sum(edge_val * (xW+b)[edge_col],
edge_row)),  computed via the identity  agg_lin = (A@x) @ W + deg x b  so the
sparse part runs on raw x, and the residual is fused as  out = a@(Wres+I)+bres.

Sharding: nodes (segment-sum destinations) are partitioned across the 8 cores
(12500 nodes each); x (host-cast to fp16) and the small dense weights are
replicated; each core processes exactly the edges whose destination lands in
its shard (host-side routing).

Per-core device algorithm (fully transposed, features on partitions):
  Phase 1, per superblock of SBW=8 destination blocks (128 dests each):
  for each of the Q=4 source chunks (int16 gather indices limit a chunk to
  <=32767 rows of x) one dma_gather pulls that chunk's edges' source rows —
  packed DENSELY in (block, slot) order, fp16, 256B per row — into an SBUF
  tile xg [128 slots, n_groups*128 feats].  Each 128-slot group g feeds one
  matmul per destination block it touches: psum[b] += xg[:, g].T @ S where
  S[slot, dest] = (iota==d)*v is built on the vector engine from per-slot
  dest-offset/value scalars (d=-1 masks slots of other blocks / padding, so
  block boundaries may fall mid-group without any padding).  Two PSUM banks
  [128 f, 4*128 d] accumulate across chunks, then flush to fp16 agg tiles.
  Gather indices and the per-instance d/v scalars stream per superblock so
  the first gathers are not serialized behind one big metadata load.
  Phase 2 (interleaved per superblock, slabs of 512 dests): psA = W.T @ agg
  + b x deg (rank-1), aT = relu(psA) in fp16; psB = (Wres+I).T @ aT + bres;
  outT[:, slab] = psB, stored transposed [128, 12544] per core; the host
  transposes + concatenates.
"""
import math

import numpy as np

import concourse.tile as tile
from concourse import bacc, mybir
from concourse.bass_utils import run_bass_kernel_spmd

F32 = mybir.dt.float32
F16 = mybir.dt.float16
I16 = mybir.dt.int16
AL = mybir.AluOpType
D = 128
P = 128
N_CORES = 8
CH = 25000        # x chunk rows (int16 gather indices => <= 32767)
SBW = 8           # destination blocks per superblock
NQ = 4            # SWDGE queues
XG_BUFS = 12      # xg pool depth
META_BUFS = 4     # idx/d/v pool depth
ALIGN = False     # 128-align each (block, chunk) segment (fewer matmuls,
                  # more gather padding) vs dense packing (opposite)


def _build(n_src, sched, repeat=1):
    n_blocks = sched["n_blocks"]
    nsh_pad = n_blocks * P
    Q = sched["Q"]
    XGW = sched["xgw"]          # fixed xg tile width (cols, feat-major)
    IC_MAX = sched["ic_max"]
    M_MAX = sched["m_max"]
    n_sb = len(sched["sbs"])

    nc = bacc.Bacc("TRN2", target_bir_lowering=False, debug=False,
                   num_swdge_queues=NQ)
    x = nc.dram_tensor("x", [n_src, D], F16, kind="ExternalInput")
    W = nc.dram_tensor("W", [D, D], F16, kind="ExternalInput")
    WresI = nc.dram_tensor("WresI", [D, D], F16, kind="ExternalInput")
    bvec = nc.dram_tensor("bvec", [1, D], F16, kind="ExternalInput")
    bres = nc.dram_tensor("bres", [1, D], F16, kind="ExternalInput")
    iotaf = nc.dram_tensor("iotaf", [P, 512], F16, kind="ExternalInput")
    deg = nc.dram_tensor("deg", [1, nsh_pad], F16, kind="ExternalInput")
    idx_t = [nc.dram_tensor(f"idx{k}", [P, sched["sbs"][k]["ic"]], I16,
                            kind="ExternalInput") for k in range(n_sb)]
    d_t = [nc.dram_tensor(f"darr{k}", [P, sched["sbs"][k]["m"]], F32,
                          kind="ExternalInput") for k in range(n_sb)]
    v_t = [nc.dram_tensor(f"varr{k}", [P, sched["sbs"][k]["m"]], F32,
                          kind="ExternalInput") for k in range(n_sb)]
    outT = nc.dram_tensor("outT", [D, nsh_pad], F32, kind="ExternalOutput")

    with tile.TileContext(nc) as tc:
        with tc.tile_pool(name="const", bufs=1) as cp:
            W_sb = cp.tile([D, D], F16)
            nc.sync.dma_start(W_sb[:], W.ap())
            WresI_sb = cp.tile([D, D], F16)
            nc.sync.dma_start(WresI_sb[:], WresI.ap())
            b_sb = cp.tile([1, D], F16)
            nc.sync.dma_start(b_sb[:], bvec.ap())
            bres_sb = cp.tile([1, D], F16)
            nc.sync.dma_start(bres_sb[:], bres.ap())
            deg_sb = cp.tile([1, nsh_pad], F16)
            nc.sync.dma_start(deg_sb[:], deg.ap())
            iota_f = cp.tile([P, 512], F16)
            nc.sync.dma_start(iota_f[:], iotaf.ap())
            ones_row = cp.tile([1, 512], F16)
            nc.vector.memset(ones_row[:], 1.0)

            for _rep in range(repeat):
                with (
                    tc.tile_pool(name="meta", bufs=META_BUFS) as mp,
                    tc.tile_pool(name="xg", bufs=XG_BUFS) as xg_pool,
                    tc.tile_pool(name="s", bufs=24) as s_pool,
                    tc.tile_pool(name="agg", bufs=2) as agg_pool,
                    tc.tile_pool(name="ot", bufs=2) as o_pool,
                    tc.tile_pool(name="ps1", bufs=6,
                                 space="PSUM") as ps1,
                    tc.tile_pool(name="psA", bufs=1, space="PSUM") as psA_pool,
                    tc.tile_pool(name="psB", bufs=1, space="PSUM") as psB_pool,
                ):
                    for sbi, sb in enumerate(sched["sbs"]):
                        nb = sb["nb"]
                        col0 = sb["col0"]
                        ic, m_sb = sb["ic"], sb["m"]
                        idx_sb = mp.tile([P, IC_MAX], I16, tag="idx",
                                         name=f"idx{sbi}")
                        nc.sync.dma_start(idx_sb[:, :ic], idx_t[sbi].ap())
                        d_sb = mp.tile([P, M_MAX], F32, tag="d",
                                       name=f"d{sbi}")
                        nc.sync.dma_start(d_sb[:, :m_sb], d_t[sbi].ap())
                        v_sb = mp.tile([P, M_MAX], F32, tag="v",
                                       name=f"v{sbi}")
                        nc.sync.dma_start(v_sb[:, :m_sb], v_t[sbi].ap())
                        xgs = []
                        for q, nidx, ioff in sb["gathers"]:
                            xg = xg_pool.tile([P, XGW], F16, tag="xg",
                                              name=f"xg{sbi}_{q}")
                            nc.gpsimd.dma_gather(
                                xg[:, :nidx].rearrange("p (g f) -> p g f", f=P),
                                x.ap()[q * CH: min(n_src, (q + 1) * CH), :],
                                idx_sb[:, ioff: ioff + nidx // 16],
                                nidx, nidx, D,
                                single_packet=(nidx <= 1024),
                                queue_num=q % NQ,
                            )
                            xgs.append(xg)
                        psbanks = [ps1.tile([P, 512], F32, tag="ps",
                                            name=f"ps{sbi}_{k}")
                                   for k in range((nb + 3) // 4)]
                        pss = [psbanks[j // 4][:, (j % 4) * P:(j % 4 + 1) * P]
                               for j in range(nb)]
                        for (qi, g, bank, c0, w, st, sp, m) in sb["instances"]:
                            S = s_pool.tile([P, 512], F16, name="S")
                            nc.vector.tensor_scalar(
                                S[:, :w], iota_f[:, c0:c0 + w],
                                d_sb[:, m:m + 1], v_sb[:, m:m + 1],
                                op0=AL.is_equal, op1=AL.mult,
                            )
                            nc.tensor.matmul(
                                out=psbanks[bank][:, c0:c0 + w],
                                lhsT=xgs[qi][:, g * P:(g + 1) * P],
                                rhs=S[:, :w],
                                start=st, stop=sp,
                            )
                        agg_sb = agg_pool.tile([P, SBW * P], F16, tag="agg",
                                               name="agg")
                        for j in range(nb):
                            nc.scalar.activation(
                                agg_sb[:, j * P:(j + 1) * P], pss[j][:],
                                mybir.ActivationFunctionType.Copy)
                        # dense head on this superblock's columns
                        for s0 in range(0, nb * P, 512):
                            w = min(512, nb * P - s0)
                            psA = psA_pool.tile([P, 512], F32, name="psA")
                            nc.tensor.matmul(out=psA[:, :w], lhsT=W_sb[:],
                                             rhs=agg_sb[:, s0:s0 + w],
                                             start=True, stop=False)
                            nc.tensor.matmul(
                                out=psA[:, :w], lhsT=b_sb[:1, :],
                                rhs=deg_sb[:1, col0 + s0: col0 + s0 + w],
                                start=False, stop=True)
                            a_t = agg_pool.tile([P, 512], F16, tag="at",
                                                name="at")
                            nc.scalar.activation(
                                a_t[:, :w], psA[:, :w],
                                mybir.ActivationFunctionType.Relu)
                            psB = psB_pool.tile([P, 512], F32, name="psB")
                            nc.tensor.matmul(out=psB[:, :w], lhsT=WresI_sb[:],
                                             rhs=a_t[:, :w],
                                             start=True, stop=False)
                            nc.tensor.matmul(out=psB[:, :w],
                                             lhsT=bres_sb[:1, :],
                                             rhs=ones_row[:1, :w],
                                             start=False, stop=True)
                            o_t = o_pool.tile([P, 512], F32, name="ot")
                            nc.scalar.activation(
                                o_t[:, :w], psB[:, :w],
                                mybir.ActivationFunctionType.Copy)
                            nc.sync.dma_start(
                                outT.ap()[:, col0 + s0: col0 + s0 + w],
                                o_t[:, :w])

    nc.compile()
    return nc


def _prep(x, W, b, Wres, bres, edge_val, edge_row, edge_col):
    x = np.asarray(x, np.float32)
    n_src = x.shape[0]
    N = n_src
    x_bf = np.ascontiguousarray(x.astype(np.float16))
    W_bf = np.ascontiguousarray(np.asarray(W, np.float32).astype(np.float16))
    WresI = np.asarray(Wres, np.float32) + np.eye(D, dtype=np.float32)
    WresI_bf = np.ascontiguousarray(WresI.astype(np.float16))
    b_bf = np.asarray(b, np.float32).reshape(1, D).astype(np.float16)
    bres_bf = np.asarray(bres, np.float32).reshape(1, D).astype(np.float16)
    edge_row = np.asarray(edge_row).astype(np.int64)
    edge_col = np.asarray(edge_col).astype(np.int64)
    edge_val = np.asarray(edge_val, np.float32)

    Q = math.ceil(n_src / CH)
    nsh = math.ceil(N / N_CORES)
    n_blocks = math.ceil(nsh / P)
    nsh_pad = n_blocks * P
    sb_list = []
    s = 0
    while s < n_blocks:
        rem = n_blocks - s
        if rem > SBW + 2:
            step = SBW
        elif rem > 4:                       # taper the tail: shorter drain
            step = rem - 4
        elif rem > 2:
            step = 2
        else:
            step = rem
        sb_list.append(list(range(s, s + step)))
        s += step
    n_sb = len(sb_list)
    NG = n_sb * Q                      # gather-group count
    blk2sb = np.empty(n_blocks, np.int64)
    blk2j = np.empty(n_blocks, np.int64)
    for si, sbl in enumerate(sb_list):
        for j, bb in enumerate(sbl):
            blk2sb[bb] = si
            blk2j[bb] = j

    # --- shard + sort edges per core, per-(sb,q) counts ---
    cores = []
    cnt = np.zeros((N_CORES, NG), np.int64)
    cnt2 = np.zeros((N_CORES, NG, SBW), np.int64)
    for c in range(N_CORES):
        lo = c * nsh
        m = (edge_row >= lo) & (edge_row < min(N, lo + nsh))
        r = edge_row[m] - lo
        ci = edge_col[m]
        v = edge_val[m]
        blk = r >> 7
        q = ci // CH
        sbid = blk2sb[blk]
        jloc = blk2j[blk]
        order = np.lexsort((jloc, q, sbid))
        r, ci, v, q, sbid, jloc = (a[order] for a in (r, ci, v, q, sbid, jloc))
        gid = sbid * Q + q
        cnt[c] = np.bincount(gid, minlength=NG)
        cnt2[c] = np.bincount(gid * SBW + jloc,
                              minlength=NG * SBW).reshape(NG, SBW)
        cores.append((r, ci, v, q, gid, jloc))

    if ALIGN:
        # each (gid, block) segment gets a fixed 128-aligned reservation:
        # no cross-block matmul instances, more gather padding
        wseg = -(-cnt2.max(axis=0) // P) * P                  # [NG, SBW]
        wseg[:, 0] = np.maximum(wseg[:, 0], P)
        seg_base = np.zeros((NG, SBW + 1), np.int64)
        np.cumsum(wseg, axis=1, out=seg_base[:, 1:])
        nidx_g = seg_base[:, -1]
        n_groups = nidx_g // P
    else:
        n_groups = np.maximum(1, -(-cnt.max(axis=0) // P))    # per gid
        nidx_g = n_groups * P
    slot_base = np.zeros(NG + 1, np.int64)
    np.cumsum(nidx_g, out=slot_base[1:])
    total_slots = int(slot_base[-1])
    G_MAX = int(n_groups.max())

    # --- instance list: per gid, which (group, block) pairs exist ---
    if ALIGN:
        Gs = seg_base[:, :-1] >> 7
        Ge = (seg_base[:, 1:] - 1) >> 7
        Ge = np.where(wseg > 0, Ge, Gs - 1)   # empty segment -> no instance
        Ge[:, 0] = np.maximum(Ge[:, 0], Gs[:, 0])
    else:
        s_cgj = np.zeros((N_CORES, NG, SBW + 1), np.int64)
        np.cumsum(cnt2, axis=2, out=s_cgj[:, :, 1:])
        starts = s_cgj[:, :, :-1]
        ends = s_cgj[:, :, 1:]
        has = cnt2 > 0
        gs = np.where(has, starts >> 7, np.iinfo(np.int64).max)
        ge = np.where(has, (ends - 1) >> 7, -1)
        Gs = gs.min(axis=0)            # [NG, SBW]
        Ge = ge.max(axis=0)
        # guarantee every block of every sb has at least one instance
        none = Ge < 0
        Gs[none] = 0
        Ge[none] = 0

    # bank-merged instances: one matmul per (gid, group, PSUM bank) covering
    # the span of blocks the group touches within that bank (wider one-hot S
    # with bank-local dest offsets instead of one matmul per block)
    NBK = (SBW + 3) // 4
    inst_span = {}                 # key -> [jmin, jmax]
    for gidx in range(NG):
        sbid = gidx // Q
        for j in range(len(sb_list[sbid])):
            for g in range(Gs[gidx, j], Ge[gidx, j] + 1):
                key = (gidx * G_MAX + g) * NBK + j // 4
                sp = inst_span.get(key)
                if sp is None:
                    inst_span[key] = [j, j]
                else:
                    sp[0] = min(sp[0], j)
                    sp[1] = max(sp[1], j)
    inst_keys = np.array(sorted(inst_span), np.int64)
    M = len(inst_keys)

    # decode instances; start/stop per PSUM bank (start zeroes a whole 2KB)
    first_of_bank = {}
    last_of_bank = {}
    inst_decode = []
    for m in range(M):
        k = int(inst_keys[m])
        bank = k % NBK
        g = (k // NBK) % G_MAX
        gidx = k // (NBK * G_MAX)
        sbid, q = gidx // Q, gidx % Q
        jmin, jmax = inst_span[(k // NBK) * NBK + bank]
        c0 = (jmin % 4) * P
        w = (jmax - jmin + 1) * P
        inst_decode.append((sbid, q, g, bank, c0, w))
        bk = (sbid, bank)
        if bk not in first_of_bank:
            first_of_bank[bk] = m
        last_of_bank[bk] = m

    # per-sb schedules with sb-local idx offsets and instance ids
    sbs_sched = []
    m_lo = np.zeros(n_sb + 1, np.int64)       # instance-id range per sb
    for m in range(M):
        m_lo[inst_decode[m][0] + 1] = m + 1
    for sbid in range(n_sb):
        nb = len(sb_list[sbid])
        gathers = []
        ioff = 0
        for q in range(Q):
            gidx = sbid * Q + q
            gathers.append((q, int(nidx_g[gidx]), ioff))
            ioff += int(nidx_g[gidx]) // 16
        instances = []
        for m in range(int(m_lo[sbid]), int(m_lo[sbid + 1])):
            s_, q_, g_, bank_, c0_, w_ = inst_decode[m]
            assert s_ == sbid
            st = first_of_bank[(sbid, bank_)] == m
            sp = last_of_bank[(sbid, bank_)] == m
            if st:
                # start=True zeroes the whole 2KB bank: write all 512 columns
                # (one-hot S leaves untouched dests at exactly 0)
                c0_, w_ = 0, 512
            instances.append((q_, g_, bank_, c0_, w_, st, sp,
                              m - int(m_lo[sbid])))
        sbs_sched.append({"nb": nb, "col0": sb_list[sbid][0] * P,
                          "gathers": gathers, "instances": instances,
                          "ic": ioff, "m": len(instances)})

    sched = {"n_blocks": n_blocks, "Q": Q, "total_slots": total_slots,
             "M": M, "xgw": G_MAX * P, "sbs": sbs_sched,
             "ic_max": max(s["ic"] for s in sbs_sched),
             "m_max": max(s["m"] for s in sbs_sched)}

    # --- per-core tensors ---
    iota_np = np.tile(np.arange(512, dtype=np.float32),
                      (P, 1)).astype(np.float16)
    in_maps = []
    for c in range(N_CORES):
        r, ci, v, q, gid, jloc = cores[c]
        if ALIGN:
            c2 = np.zeros(NG * SBW + 1, np.int64)
            np.cumsum(cnt2[c].reshape(-1), out=c2[1:])
            rank = np.arange(len(r), dtype=np.int64) - c2[gid * SBW + jloc]
            slot = slot_base[gid] + seg_base[gid, jloc] + rank
        else:
            gstart = np.zeros(NG + 1, np.int64)
            np.cumsum(cnt[c], out=gstart[1:])
            rank = np.arange(len(r), dtype=np.int64) - gstart[gid]
            slot = slot_base[gid] + rank

        idx16 = np.zeros(total_slots, np.int16)
        idx16[slot] = (ci - q * CH).astype(np.int16)

        NBK = (SBW + 3) // 4
        ke = (gid * G_MAX + ((slot - slot_base[gid]) >> 7)) * NBK + jloc // 4
        me = np.searchsorted(inst_keys, ke)
        assert (inst_keys[me] == ke).all()
        d_all = np.full((P, M), -1.0, np.float32)
        v_all = np.zeros((P, M), np.float32)
        d_all[slot & 127, me] = ((jloc % 4) * P + (r & 127)).astype(np.float32)
        v_all[slot & 127, me] = v

        degv = np.zeros(nsh_pad, np.float32)
        lo = c * nsh
        hi = min(N, lo + nsh)
        degv[:hi - lo] = np.bincount(r, weights=v, minlength=hi - lo
                                     ).astype(np.float32)[:hi - lo]
        im = {
            "x": x_bf, "W": W_bf, "WresI": WresI_bf, "bvec": b_bf,
            "bres": bres_bf, "iotaf": iota_np,
            "deg": degv.astype(np.float16).reshape(1, nsh_pad),
        }
        for sbid in range(n_sb):
            g0 = sbid * Q
            sl0, sl1 = int(slot_base[g0]), int(slot_base[g0 + Q])
            seg = idx16[sl0:sl1]
            im[f"idx{sbid}"] = np.tile(
                np.ascontiguousarray(seg.reshape(len(seg) // 16, 16).T),
                (8, 1))
            a0, a1 = int(m_lo[sbid]), int(m_lo[sbid + 1])
            im[f"darr{sbid}"] = np.ascontiguousarray(d_all[:, a0:a1])
            im[f"varr{sbid}"] = np.ascontiguousarray(v_all[:, a0:a1])
        in_maps.append(im)
    meta = dict(N=N, nsh=nsh, n_blocks=n_blocks, nsh_pad=nsh_pad, Q=Q)
    return in_maps, meta, sched


def kernel(x, W, b, Wres, bres, edge_val, edge_row, edge_col):
    in_maps, meta, sched = _prep(x, W, b, Wres, bres,
                                 edge_val, edge_row, edge_col)
    nc = _build(np.asarray(x).shape[0], sched)
    res = run_bass_kernel_spmd(nc, in_maps, core_ids=list(range(N_CORES)))
    N, nsh = meta["N"], meta["nsh"]
    out = np.empty((N, D), np.float32)
    for c in range(N_CORES):
        lo = c * nsh
        hi = min(N, lo + nsh)
        out[lo:hi] = res.results[c]["outT"].T[: hi - lo]
    return out


# revision 39
# speedup vs baseline: 1.0428x; 1.0428x over previous
"""GCNConv (graph message passing) on 8 Trainium2 NeuronCores — Bass/Tile.

out = a + (a @ Wres + bres),  a = relu(segment_sum(edge_val * (xW+b)[edge_col],
edge_row)),  computed via the identity  agg_lin = (A@x) @ W + deg x b  so the
sparse part runs on raw x, and the residual is fused as  out = a@(Wres+I)+bres.

Sharding: nodes (segment-sum destinations) are partitioned across the 8 cores
(12500 nodes each); x (host-cast to fp16) and the small dense weights are
replicated; each core processes exactly the edges whose destination lands in
its shard (host-side routing).

Per-core device algorithm (fully transposed, features on partitions):
  Phase 1, per superblock of SBW=8 destination blocks (128 dests each):
  for each of the Q=4 source chunks (int16 gather indices limit a chunk to
  <=32767 rows of x) one dma_gather pulls that chunk's edges' source rows —
  packed DENSELY in (block, slot) order, fp16, 256B per row — into an SBUF
  tile xg [128 slots, n_groups*128 feats].  Each 128-slot group g feeds one
  matmul per destination block it touches: psum[b] += xg[:, g].T @ S where
  S[slot, dest] = (iota==d)*v is built on the vector engine from per-slot
  dest-offset/value scalars (d=-1 masks slots of other blocks / padding, so
  block boundaries may fall mid-group without any padding).  Two PSUM banks
  [128 f, 4*128 d] accumulate across chunks, then flush to fp16 agg tiles.
  Gather indices and the per-instance d/v scalars stream per superblock so
  the first gathers are not serialized behind one big metadata load.
  Phase 2 (interleaved per superblock, slabs of 512 dests): psA = W.T @ agg
  + b x deg (rank-1), aT = relu(psA) in fp16; psB = (Wres+I).T @ aT + bres;
  outT[:, slab] = psB, stored transposed [128, 12544] per core; the host
  transposes + concatenates.
"""
import math

import numpy as np

import concourse.tile as tile
from concourse import bacc, mybir
from concourse.bass_utils import run_bass_kernel_spmd

F32 = mybir.dt.float32
F16 = mybir.dt.float16
I16 = mybir.dt.int16
AL = mybir.AluOpType
D = 128
P = 128
N_CORES = 8
CH = 25000        # x chunk rows (int16 gather indices => <= 32767)
SBW = 8           # destination blocks per superblock
NQ = 4            # SWDGE queues
S_POOL_EVERY = 0  # every Nth one-hot S is built on gpsimd instead of DVE
                  # (0 = never) to relieve the DVE sequencer
XG_BUFS = 12      # xg pool depth
META_BUFS = 4     # idx/d/v pool depth
ALIGN = False     # 128-align each (block, chunk) segment (fewer matmuls,
                  # more gather padding) vs dense packing (opposite)


def _build(n_src, sched, repeat=1):
    n_blocks = sched["n_blocks"]
    nsh_pad = n_blocks * P
    Q = sched["Q"]
    XGW = sched["xgw"]          # fixed xg tile width (cols, feat-major)
    IC_MAX = sched["ic_max"]
    M_MAX = sched["m_max"]
    n_sb = len(sched["sbs"])

    nc = bacc.Bacc("TRN2", target_bir_lowering=False, debug=False,
                   num_swdge_queues=NQ)
    x = nc.dram_tensor("x", [n_src, D], F16, kind="ExternalInput")
    cpack = nc.dram_tensor("cpack", [P, 768], F16, kind="ExternalInput")
    rpack = nc.dram_tensor("rpack", [1, 256 + nsh_pad], F16,
                           kind="ExternalInput")
    idx_t = [nc.dram_tensor(f"idx{k}", [P, sched["sbs"][k]["ic"]], I16,
                            kind="ExternalInput") for k in range(n_sb)]
    d_t = [nc.dram_tensor(f"darr{k}", [P, sched["sbs"][k]["m"]], F32,
                          kind="ExternalInput") for k in range(n_sb)]
    v_t = [nc.dram_tensor(f"varr{k}", [P, sched["sbs"][k]["m"]], F32,
                          kind="ExternalInput") for k in range(n_sb)]
    outT = nc.dram_tensor("outT", [D, nsh_pad], F16, kind="ExternalOutput")

    with tile.TileContext(nc) as tc:
        with tc.tile_pool(name="const", bufs=1) as cp:
            cpk = cp.tile([P, 768], F16)
            nc.sync.dma_start(cpk[:], cpack.ap())
            W_sb = cpk[:, 0:D]
            WresI_sb = cpk[:, D:2 * D]
            iota_f = cpk[:, 2 * D:2 * D + 512]
            rpk = cp.tile([1, 256 + nsh_pad], F16)
            nc.sync.dma_start(rpk[:], rpack.ap())
            b_sb = rpk[:1, 0:D]
            bres_sb = rpk[:1, D:2 * D]
            deg_sb = rpk[:1, 256:]
            ones_row = cp.tile([1, 512], F16)
            nc.vector.memset(ones_row[:], 1.0)

            for _rep in range(repeat):
                with (
                    tc.tile_pool(name="meta", bufs=META_BUFS) as mp,
                    tc.tile_pool(name="xg", bufs=XG_BUFS) as xg_pool,
                    tc.tile_pool(name="s", bufs=24) as s_pool,
                    tc.tile_pool(name="agg", bufs=2) as agg_pool,
                    tc.tile_pool(name="ot", bufs=2) as o_pool,
                    tc.tile_pool(name="ps1", bufs=6,
                                 space="PSUM") as ps1,
                    tc.tile_pool(name="psA", bufs=1, space="PSUM") as psA_pool,
                    tc.tile_pool(name="psB", bufs=1, space="PSUM") as psB_pool,
                ):
                    for sbi, sb in enumerate(sched["sbs"]):
                        nb = sb["nb"]
                        col0 = sb["col0"]
                        ic, m_sb = sb["ic"], sb["m"]
                        idx_sb = mp.tile([P, IC_MAX], I16, tag="idx",
                                         name=f"idx{sbi}")
                        nc.sync.dma_start(idx_sb[:, :ic], idx_t[sbi].ap())
                        d_sb = mp.tile([P, M_MAX], F32, tag="d",
                                       name=f"d{sbi}")
                        nc.sync.dma_start(d_sb[:, :m_sb], d_t[sbi].ap())
                        v_sb = mp.tile([P, M_MAX], F32, tag="v",
                                       name=f"v{sbi}")
                        nc.sync.dma_start(v_sb[:, :m_sb], v_t[sbi].ap())
                        xgs = []
                        for q, nidx, ioff in sb["gathers"]:
                            xg = xg_pool.tile([P, XGW], F16, tag="xg",
                                              name=f"xg{sbi}_{q}")
                            nc.gpsimd.dma_gather(
                                xg[:, :nidx].rearrange("p (g f) -> p g f", f=P),
                                x.ap()[q * CH: min(n_src, (q + 1) * CH), :],
                                idx_sb[:, ioff: ioff + nidx // 16],
                                nidx, nidx, D,
                                single_packet=(nidx <= 1024),
                                queue_num=q % NQ,
                            )
                            xgs.append(xg)
                        psbanks = [ps1.tile([P, 512], F32, tag="ps",
                                            name=f"ps{sbi}_{k}")
                                   for k in range((nb + 3) // 4)]
                        pss = [psbanks[j // 4][:, (j % 4) * P:(j % 4 + 1) * P]
                               for j in range(nb)]
                        squad, qslot = None, 4
                        for ii, (qi, g, bank, c0, w, st, sp, m) in \
                                enumerate(sb["instances"]):
                            eng = (nc.gpsimd if S_POOL_EVERY and
                                   ii % S_POOL_EVERY == S_POOL_EVERY - 1
                                   else nc.vector)
                            if w == P:
                                # pack 4 narrow S's per pool tile: 4x fewer
                                # WAR semaphore waits on the DVE sequencer
                                if qslot == 4:
                                    squad = s_pool.tile([P, 512], F16,
                                                        name="S")
                                    qslot = 0
                                Sap = squad[:, qslot * P:(qslot + 1) * P]
                                qslot += 1
                            else:
                                Sap = s_pool.tile([P, 512], F16,
                                                  name="S")[:, :w]
                            eng.tensor_scalar(
                                Sap, iota_f[:, c0:c0 + w],
                                d_sb[:, m:m + 1], v_sb[:, m:m + 1],
                                op0=AL.is_equal, op1=AL.mult,
                            )
                            nc.tensor.matmul(
                                out=psbanks[bank][:, c0:c0 + w],
                                lhsT=xgs[qi][:, g * P:(g + 1) * P],
                                rhs=Sap,
                                start=st, stop=sp,
                            )
                        agg_sb = agg_pool.tile([P, SBW * P], F16, tag="agg",
                                               name="agg")
                        for k in range((nb + 3) // 4):
                            bw = min(512, nb * P - k * 512)
                            nc.scalar.activation(
                                agg_sb[:, k * 512:k * 512 + bw],
                                psbanks[k][:, :bw],
                                mybir.ActivationFunctionType.Copy)
                        # dense head on this superblock's columns
                        for s0 in range(0, nb * P, 512):
                            w = min(512, nb * P - s0)
                            psA = psA_pool.tile([P, 512], F32, name="psA")
                            nc.tensor.matmul(out=psA[:, :w], lhsT=W_sb,
                                             rhs=agg_sb[:, s0:s0 + w],
                                             start=True, stop=False)
                            nc.tensor.matmul(
                                out=psA[:, :w], lhsT=b_sb,
                                rhs=deg_sb[:, col0 + s0: col0 + s0 + w],
                                start=False, stop=True)
                            a_t = agg_pool.tile([P, 512], F16, tag="at",
                                                name="at")
                            nc.scalar.activation(
                                a_t[:, :w], psA[:, :w],
                                mybir.ActivationFunctionType.Relu)
                            psB = psB_pool.tile([P, 512], F32, name="psB")
                            nc.tensor.matmul(out=psB[:, :w], lhsT=WresI_sb,
                                             rhs=a_t[:, :w],
                                             start=True, stop=False)
                            nc.tensor.matmul(out=psB[:, :w],
                                             lhsT=bres_sb,
                                             rhs=ones_row[:1, :w],
                                             start=False, stop=True)
                            o_t = o_pool.tile([P, 512], F16, name="ot")
                            nc.scalar.activation(
                                o_t[:, :w], psB[:, :w],
                                mybir.ActivationFunctionType.Copy)
                            nc.sync.dma_start(
                                outT.ap()[:, col0 + s0: col0 + s0 + w],
                                o_t[:, :w])

    nc.compile()
    return nc


def _prep(x, W, b, Wres, bres, edge_val, edge_row, edge_col):
    x = np.asarray(x, np.float32)
    n_src = x.shape[0]
    N = n_src
    x_bf = np.ascontiguousarray(x.astype(np.float16))
    W_bf = np.ascontiguousarray(np.asarray(W, np.float32).astype(np.float16))
    WresI = np.asarray(Wres, np.float32) + np.eye(D, dtype=np.float32)
    WresI_bf = np.ascontiguousarray(WresI.astype(np.float16))
    b_bf = np.asarray(b, np.float32).reshape(1, D).astype(np.float16)
    bres_bf = np.asarray(bres, np.float32).reshape(1, D).astype(np.float16)
    edge_row = np.asarray(edge_row).astype(np.int64)
    edge_col = np.asarray(edge_col).astype(np.int64)
    edge_val = np.asarray(edge_val, np.float32)

    Q = math.ceil(n_src / CH)
    nsh = math.ceil(N / N_CORES)
    n_blocks = math.ceil(nsh / P)
    nsh_pad = n_blocks * P
    sb_list = []
    s = 0
    while s < n_blocks:
        rem = n_blocks - s
        if rem > SBW + 2:
            step = SBW
        elif rem > 4:                       # taper the tail: shorter drain
            step = rem - 4
        elif rem > 2:
            step = 2
        else:
            step = rem
        sb_list.append(list(range(s, s + step)))
        s += step
    n_sb = len(sb_list)
    NG = n_sb * Q                      # gather-group count
    blk2sb = np.empty(n_blocks, np.int64)
    blk2j = np.empty(n_blocks, np.int64)
    for si, sbl in enumerate(sb_list):
        for j, bb in enumerate(sbl):
            blk2sb[bb] = si
            blk2j[bb] = j

    # --- shard + sort edges per core, per-(sb,q) counts ---
    cores = []
    cnt = np.zeros((N_CORES, NG), np.int64)
    cnt2 = np.zeros((N_CORES, NG, SBW), np.int64)
    for c in range(N_CORES):
        lo = c * nsh
        m = (edge_row >= lo) & (edge_row < min(N, lo + nsh))
        r = edge_row[m] - lo
        ci = edge_col[m]
        v = edge_val[m]
        blk = r >> 7
        q = ci // CH
        sbid = blk2sb[blk]
        jloc = blk2j[blk]
        order = np.lexsort((jloc, q, sbid))
        r, ci, v, q, sbid, jloc = (a[order] for a in (r, ci, v, q, sbid, jloc))
        gid = sbid * Q + q
        cnt[c] = np.bincount(gid, minlength=NG)
        cnt2[c] = np.bincount(gid * SBW + jloc,
                              minlength=NG * SBW).reshape(NG, SBW)
        cores.append((r, ci, v, q, gid, jloc))

    if ALIGN:
        # each (gid, block) segment gets a fixed 128-aligned reservation:
        # no cross-block matmul instances, more gather padding
        wseg = -(-cnt2.max(axis=0) // P) * P                  # [NG, SBW]
        wseg[:, 0] = np.maximum(wseg[:, 0], P)
        seg_base = np.zeros((NG, SBW + 1), np.int64)
        np.cumsum(wseg, axis=1, out=seg_base[:, 1:])
        nidx_g = seg_base[:, -1]
        n_groups = nidx_g // P
    else:
        n_groups = np.maximum(1, -(-cnt.max(axis=0) // P))    # per gid
        nidx_g = n_groups * P
    slot_base = np.zeros(NG + 1, np.int64)
    np.cumsum(nidx_g, out=slot_base[1:])
    total_slots = int(slot_base[-1])
    G_MAX = int(n_groups.max())

    # --- instance list: per gid, which (group, block) pairs exist ---
    if ALIGN:
        Gs = seg_base[:, :-1] >> 7
        Ge = (seg_base[:, 1:] - 1) >> 7
        Ge = np.where(wseg > 0, Ge, Gs - 1)   # empty segment -> no instance
        Ge[:, 0] = np.maximum(Ge[:, 0], Gs[:, 0])
    else:
        s_cgj = np.zeros((N_CORES, NG, SBW + 1), np.int64)
        np.cumsum(cnt2, axis=2, out=s_cgj[:, :, 1:])
        starts = s_cgj[:, :, :-1]
        ends = s_cgj[:, :, 1:]
        has = cnt2 > 0
        gs = np.where(has, starts >> 7, np.iinfo(np.int64).max)
        ge = np.where(has, (ends - 1) >> 7, -1)
        Gs = gs.min(axis=0)            # [NG, SBW]
        Ge = ge.max(axis=0)
        # guarantee every block of every sb has at least one instance
        none = Ge < 0
        Gs[none] = 0
        Ge[none] = 0

    # bank-merged instances: one matmul per (gid, group, PSUM bank) covering
    # the span of blocks the group touches within that bank (wider one-hot S
    # with bank-local dest offsets instead of one matmul per block)
    NBK = (SBW + 3) // 4
    inst_span = {}                 # key -> [jmin, jmax]
    for gidx in range(NG):
        sbid = gidx // Q
        for j in range(len(sb_list[sbid])):
            for g in range(Gs[gidx, j], Ge[gidx, j] + 1):
                key = (gidx * G_MAX + g) * NBK + j // 4
                sp = inst_span.get(key)
                if sp is None:
                    inst_span[key] = [j, j]
                else:
                    sp[0] = min(sp[0], j)
                    sp[1] = max(sp[1], j)
    inst_keys = np.array(sorted(inst_span), np.int64)
    M = len(inst_keys)

    # decode instances; start/stop per PSUM bank (start zeroes a whole 2KB)
    first_of_bank = {}
    last_of_bank = {}
    inst_decode = []
    for m in range(M):
        k = int(inst_keys[m])
        bank = k % NBK
        g = (k // NBK) % G_MAX
        gidx = k // (NBK * G_MAX)
        sbid, q = gidx // Q, gidx % Q
        jmin, jmax = inst_span[(k // NBK) * NBK + bank]
        c0 = (jmin % 4) * P
        w = (jmax - jmin + 1) * P
        inst_decode.append((sbid, q, g, bank, c0, w))
        bk = (sbid, bank)
        if bk not in first_of_bank:
            first_of_bank[bk] = m
        last_of_bank[bk] = m

    # per-sb schedules with sb-local idx offsets and instance ids
    sbs_sched = []
    m_lo = np.zeros(n_sb + 1, np.int64)       # instance-id range per sb
    for m in range(M):
        m_lo[inst_decode[m][0] + 1] = m + 1
    for sbid in range(n_sb):
        nb = len(sb_list[sbid])
        gathers = []
        ioff = 0
        for q in range(Q):
            gidx = sbid * Q + q
            gathers.append((q, int(nidx_g[gidx]), ioff))
            ioff += int(nidx_g[gidx]) // 16
        instances = []
        for m in range(int(m_lo[sbid]), int(m_lo[sbid + 1])):
            s_, q_, g_, bank_, c0_, w_ = inst_decode[m]
            assert s_ == sbid
            st = first_of_bank[(sbid, bank_)] == m
            sp = last_of_bank[(sbid, bank_)] == m
            if st:
                # start=True zeroes the whole 2KB bank: write all 512 columns
                # (one-hot S leaves untouched dests at exactly 0)
                c0_, w_ = 0, 512
            instances.append((q_, g_, bank_, c0_, w_, st, sp,
                              m - int(m_lo[sbid])))
        sbs_sched.append({"nb": nb, "col0": sb_list[sbid][0] * P,
                          "gathers": gathers, "instances": instances,
                          "ic": ioff, "m": len(instances)})

    sched = {"n_blocks": n_blocks, "Q": Q, "total_slots": total_slots,
             "M": M, "xgw": G_MAX * P, "sbs": sbs_sched,
             "ic_max": max(s["ic"] for s in sbs_sched),
             "m_max": max(s["m"] for s in sbs_sched)}

    # --- per-core tensors ---
    iota_np = np.tile(np.arange(512, dtype=np.float32),
                      (P, 1)).astype(np.float16)
    in_maps = []
    for c in range(N_CORES):
        r, ci, v, q, gid, jloc = cores[c]
        if ALIGN:
            c2 = np.zeros(NG * SBW + 1, np.int64)
            np.cumsum(cnt2[c].reshape(-1), out=c2[1:])
            rank = np.arange(len(r), dtype=np.int64) - c2[gid * SBW + jloc]
            slot = slot_base[gid] + seg_base[gid, jloc] + rank
        else:
            gstart = np.zeros(NG + 1, np.int64)
            np.cumsum(cnt[c], out=gstart[1:])
            rank = np.arange(len(r), dtype=np.int64) - gstart[gid]
            slot = slot_base[gid] + rank

        idx16 = np.zeros(total_slots, np.int16)
        idx16[slot] = (ci - q * CH).astype(np.int16)

        NBK = (SBW + 3) // 4
        ke = (gid * G_MAX + ((slot - slot_base[gid]) >> 7)) * NBK + jloc // 4
        me = np.searchsorted(inst_keys, ke)
        assert (inst_keys[me] == ke).all()
        d_all = np.full((P, M), -1.0, np.float32)
        v_all = np.zeros((P, M), np.float32)
        d_all[slot & 127, me] = ((jloc % 4) * P + (r & 127)).astype(np.float32)
        v_all[slot & 127, me] = v

        degv = np.zeros(nsh_pad, np.float32)
        lo = c * nsh
        hi = min(N, lo + nsh)
        degv[:hi - lo] = np.bincount(r, weights=v, minlength=hi - lo
                                     ).astype(np.float32)[:hi - lo]
        cpk = np.concatenate([W_bf, WresI_bf, iota_np], axis=1)
        rpk = np.concatenate(
            [b_bf.ravel(), bres_bf.ravel(), degv.astype(np.float16)]
        ).reshape(1, 256 + nsh_pad)
        im = {
            "x": x_bf, "cpack": np.ascontiguousarray(cpk),
            "rpack": np.ascontiguousarray(rpk),
        }
        for sbid in range(n_sb):
            g0 = sbid * Q
            sl0, sl1 = int(slot_base[g0]), int(slot_base[g0 + Q])
            seg = idx16[sl0:sl1]
            im[f"idx{sbid}"] = np.tile(
                np.ascontiguousarray(seg.reshape(len(seg) // 16, 16).T),
                (8, 1))
            a0, a1 = int(m_lo[sbid]), int(m_lo[sbid + 1])
            im[f"darr{sbid}"] = np.ascontiguousarray(d_all[:, a0:a1])
            im[f"varr{sbid}"] = np.ascontiguousarray(v_all[:, a0:a1])
        in_maps.append(im)
    meta = dict(N=N, nsh=nsh, n_blocks=n_blocks, nsh_pad=nsh_pad, Q=Q)
    return in_maps, meta, sched


def kernel(x, W, b, Wres, bres, edge_val, edge_row, edge_col):
    in_maps, meta, sched = _prep(x, W, b, Wres, bres,
                                 edge_val, edge_row, edge_col)
    nc = _build(np.asarray(x).shape[0], sched)
    res = run_bass_kernel_spmd(nc, in_maps, core_ids=list(range(N_CORES)))
    N, nsh = meta["N"], meta["nsh"]
    out = np.empty((N, D), np.float32)
    for c in range(N_CORES):
        lo = c * nsh
        hi = min(N, lo + nsh)
        out[lo:hi] = res.results[c]["outT"].T[: hi - lo].astype(np.float32)
    return out


# revision 40
# speedup vs baseline: 1.0556x; 1.0123x over previous
"""GCNConv (graph message passing) on 8 Trainium2 NeuronCores — Bass/Tile.

out = a + (a @ Wres + bres),  a = relu(segment_sum(edge_val * (xW+b)[edge_col],
edge_row)),  computed via the identity  agg_lin = (A@x) @ W + deg x b  so the
sparse part runs on raw x, and the residual is fused as  out = a@(Wres+I)+bres.

Sharding: nodes (segment-sum destinations) are partitioned across the 8 cores
(12500 nodes each); x (host-cast to fp16) and the small dense weights are
replicated; each core processes exactly the edges whose destination lands in
its shard (host-side routing).

Per-core device algorithm (fully transposed, features on partitions):
  Phase 1, per superblock of SBW=8 destination blocks (128 dests each):
  for each of the Q=4 source chunks (int16 gather indices limit a chunk to
  <=32767 rows of x) one dma_gather pulls that chunk's edges' source rows —
  packed DENSELY in (block, slot) order, fp16, 256B per row — into an SBUF
  tile xg [128 slots, n_groups*128 feats].  Each 128-slot group g feeds one
  matmul per destination block it touches: psum[b] += xg[:, g].T @ S where
  S[slot, dest] = (iota==d)*v is built on the vector engine from per-slot
  dest-offset/value scalars (d=-1 masks slots of other blocks / padding, so
  block boundaries may fall mid-group without any padding).  Two PSUM banks
  [128 f, 4*128 d] accumulate across chunks, then flush to fp16 agg tiles.
  Gather indices and the per-instance d/v scalars stream per superblock so
  the first gathers are not serialized behind one big metadata load.
  Phase 2 (interleaved per superblock, slabs of 512 dests): psA = W.T @ agg
  + b x deg (rank-1), aT = relu(psA) in fp16; psB = (Wres+I).T @ aT + bres;
  outT[:, slab] = psB, stored transposed [128, 12544] per core; the host
  transposes + concatenates.
"""
import math

import numpy as np

import concourse.tile as tile
from concourse import bacc, mybir
from concourse.bass_utils import run_bass_kernel_spmd

F32 = mybir.dt.float32
F16 = mybir.dt.float16
I16 = mybir.dt.int16
AL = mybir.AluOpType
D = 128
P = 128
N_CORES = 8
CH = 25000        # x chunk rows (int16 gather indices => <= 32767)
SBW = 8           # destination blocks per superblock
NQ = 4            # SWDGE queues
S_POOL_EVERY = 0  # every Nth one-hot S is built on gpsimd instead of DVE
                  # (0 = never) to relieve the DVE sequencer
XG_BUFS = 13      # xg pool depth
META_BUFS = 5     # idx/d/v pool depth
ALIGN = False     # 128-align each (block, chunk) segment (fewer matmuls,
                  # more gather padding) vs dense packing (opposite)


def _build(n_src, sched, repeat=1):
    n_blocks = sched["n_blocks"]
    nsh_pad = n_blocks * P
    Q = sched["Q"]
    XGW = sched["xgw"]          # fixed xg tile width (cols, feat-major)
    IC_MAX = sched["ic_max"]
    M_MAX = sched["m_max"]
    n_sb = len(sched["sbs"])

    nc = bacc.Bacc("TRN2", target_bir_lowering=False, debug=False,
                   num_swdge_queues=NQ)
    x = nc.dram_tensor("x", [n_src, D], F16, kind="ExternalInput")
    cpack = nc.dram_tensor("cpack", [P, 768], F16, kind="ExternalInput")
    rpack = nc.dram_tensor("rpack", [1, 256 + nsh_pad], F16,
                           kind="ExternalInput")
    idx_t = [nc.dram_tensor(f"idx{k}", [P, sched["sbs"][k]["ic"]], I16,
                            kind="ExternalInput") for k in range(n_sb)]
    d_t = [nc.dram_tensor(f"darr{k}", [P, sched["sbs"][k]["m"]], F32,
                          kind="ExternalInput") for k in range(n_sb)]
    v_t = [nc.dram_tensor(f"varr{k}", [P, sched["sbs"][k]["m"]], F32,
                          kind="ExternalInput") for k in range(n_sb)]
    outT = nc.dram_tensor("outT", [D, nsh_pad], F16, kind="ExternalOutput")

    with tile.TileContext(nc) as tc:
        with tc.tile_pool(name="const", bufs=1) as cp:
            cpk = cp.tile([P, 768], F16)
            nc.sync.dma_start(cpk[:], cpack.ap())
            W_sb = cpk[:, 0:D]
            WresI_sb = cpk[:, D:2 * D]
            iota_f = cpk[:, 2 * D:2 * D + 512]
            rpk = cp.tile([1, 256 + nsh_pad], F16)
            nc.sync.dma_start(rpk[:], rpack.ap())
            b_sb = rpk[:1, 0:D]
            bres_sb = rpk[:1, D:2 * D]
            deg_sb = rpk[:1, 256:]
            ones_row = cp.tile([1, 512], F16)
            nc.vector.memset(ones_row[:], 1.0)

            for _rep in range(repeat):
                with (
                    tc.tile_pool(name="meta", bufs=META_BUFS) as mp,
                    tc.tile_pool(name="xg", bufs=XG_BUFS) as xg_pool,
                    tc.tile_pool(name="s", bufs=24) as s_pool,
                    tc.tile_pool(name="agg", bufs=2) as agg_pool,
                    tc.tile_pool(name="ot", bufs=2) as o_pool,
                    tc.tile_pool(name="ps1", bufs=6,
                                 space="PSUM") as ps1,
                    tc.tile_pool(name="psA", bufs=1, space="PSUM") as psA_pool,
                    tc.tile_pool(name="psB", bufs=1, space="PSUM") as psB_pool,
                ):
                    for sbi, sb in enumerate(sched["sbs"]):
                        nb = sb["nb"]
                        col0 = sb["col0"]
                        ic, m_sb = sb["ic"], sb["m"]
                        idx_sb = mp.tile([P, IC_MAX], I16, tag="idx",
                                         name=f"idx{sbi}")
                        nc.sync.dma_start(idx_sb[:, :ic], idx_t[sbi].ap())
                        d_sb = mp.tile([P, M_MAX], F32, tag="d",
                                       name=f"d{sbi}")
                        nc.sync.dma_start(d_sb[:, :m_sb], d_t[sbi].ap())
                        v_sb = mp.tile([P, M_MAX], F32, tag="v",
                                       name=f"v{sbi}")
                        nc.sync.dma_start(v_sb[:, :m_sb], v_t[sbi].ap())
                        xgs = []
                        for q, nidx, ioff in sb["gathers"]:
                            xg = xg_pool.tile([P, XGW], F16, tag="xg",
                                              name=f"xg{sbi}_{q}")
                            nc.gpsimd.dma_gather(
                                xg[:, :nidx].rearrange("p (g f) -> p g f", f=P),
                                x.ap()[q * CH: min(n_src, (q + 1) * CH), :],
                                idx_sb[:, ioff: ioff + nidx // 16],
                                nidx, nidx, D,
                                single_packet=(nidx <= 1024),
                                queue_num=q % NQ,
                            )
                            xgs.append(xg)
                        psbanks = [ps1.tile([P, 512], F32, tag="ps",
                                            name=f"ps{sbi}_{k}")
                                   for k in range((nb + 3) // 4)]
                        pss = [psbanks[j // 4][:, (j % 4) * P:(j % 4 + 1) * P]
                               for j in range(nb)]
                        squad, qslot = None, 4
                        for ii, (qi, g, bank, c0, w, st, sp, m) in \
                                enumerate(sb["instances"]):
                            eng = (nc.gpsimd if S_POOL_EVERY and
                                   ii % S_POOL_EVERY == S_POOL_EVERY - 1
                                   else nc.vector)
                            if w == P:
                                # pack 4 narrow S's per pool tile: 4x fewer
                                # WAR semaphore waits on the DVE sequencer
                                if qslot == 4:
                                    squad = s_pool.tile([P, 512], F16,
                                                        name="S")
                                    qslot = 0
                                Sap = squad[:, qslot * P:(qslot + 1) * P]
                                qslot += 1
                            else:
                                Sap = s_pool.tile([P, 512], F16,
                                                  name="S")[:, :w]
                            eng.tensor_scalar(
                                Sap, iota_f[:, c0:c0 + w],
                                d_sb[:, m:m + 1], v_sb[:, m:m + 1],
                                op0=AL.is_equal, op1=AL.mult,
                            )
                            nc.tensor.matmul(
                                out=psbanks[bank][:, c0:c0 + w],
                                lhsT=xgs[qi][:, g * P:(g + 1) * P],
                                rhs=Sap,
                                start=st, stop=sp,
                            )
                        agg_sb = agg_pool.tile([P, SBW * P], F16, tag="agg",
                                               name="agg")
                        for k in range((nb + 3) // 4):
                            bw = min(512, nb * P - k * 512)
                            nc.scalar.activation(
                                agg_sb[:, k * 512:k * 512 + bw],
                                psbanks[k][:, :bw],
                                mybir.ActivationFunctionType.Copy)
                        # dense head on this superblock's columns
                        for s0 in range(0, nb * P, 512):
                            w = min(512, nb * P - s0)
                            psA = psA_pool.tile([P, 512], F32, name="psA")
                            nc.tensor.matmul(out=psA[:, :w], lhsT=W_sb,
                                             rhs=agg_sb[:, s0:s0 + w],
                                             start=True, stop=False)
                            nc.tensor.matmul(
                                out=psA[:, :w], lhsT=b_sb,
                                rhs=deg_sb[:, col0 + s0: col0 + s0 + w],
                                start=False, stop=True)
                            a_t = agg_pool.tile([P, 512], F16, tag="at",
                                                name="at")
                            nc.scalar.activation(
                                a_t[:, :w], psA[:, :w],
                                mybir.ActivationFunctionType.Relu)
                            psB = psB_pool.tile([P, 512], F32, name="psB")
                            nc.tensor.matmul(out=psB[:, :w], lhsT=WresI_sb,
                                             rhs=a_t[:, :w],
                                             start=True, stop=False)
                            nc.tensor.matmul(out=psB[:, :w],
                                             lhsT=bres_sb,
                                             rhs=ones_row[:1, :w],
                                             start=False, stop=True)
                            o_t = o_pool.tile([P, 512], F16, name="ot")
                            nc.scalar.activation(
                                o_t[:, :w], psB[:, :w],
                                mybir.ActivationFunctionType.Copy)
                            nc.sync.dma_start(
                                outT.ap()[:, col0 + s0: col0 + s0 + w],
                                o_t[:, :w])

    nc.compile()
    return nc


def _prep(x, W, b, Wres, bres, edge_val, edge_row, edge_col):
    x = np.asarray(x, np.float32)
    n_src = x.shape[0]
    N = n_src
    x_bf = np.ascontiguousarray(x.astype(np.float16))
    W_bf = np.ascontiguousarray(np.asarray(W, np.float32).astype(np.float16))
    WresI = np.asarray(Wres, np.float32) + np.eye(D, dtype=np.float32)
    WresI_bf = np.ascontiguousarray(WresI.astype(np.float16))
    b_bf = np.asarray(b, np.float32).reshape(1, D).astype(np.float16)
    bres_bf = np.asarray(bres, np.float32).reshape(1, D).astype(np.float16)
    edge_row = np.asarray(edge_row).astype(np.int64)
    edge_col = np.asarray(edge_col).astype(np.int64)
    edge_val = np.asarray(edge_val, np.float32)

    Q = math.ceil(n_src / CH)
    nsh = math.ceil(N / N_CORES)
    n_blocks = math.ceil(nsh / P)
    nsh_pad = n_blocks * P
    # taper BOTH ends: small superblocks first (compute starts sooner) and
    # last (short drain after the final gather); full-width in the middle
    steps = []
    rem = n_blocks
    for head in (2, 3):
        if rem > SBW + 6:
            steps.append(head)
            rem -= head
    tail = []
    for t in (1, 2, 3):
        if rem > SBW + 6:
            tail.append(t)
            rem -= t
    while rem > 0:
        if rem > SBW + 2:
            s_ = SBW
        elif rem > 4:
            s_ = rem - 4
        elif rem > 2:
            s_ = 2
        else:
            s_ = rem
        steps.append(s_)
        rem -= s_
    steps.extend(reversed(tail))
    sb_list = []
    s = 0
    for step in steps:
        sb_list.append(list(range(s, s + step)))
        s += step
    assert s == n_blocks
    n_sb = len(sb_list)
    NG = n_sb * Q                      # gather-group count
    blk2sb = np.empty(n_blocks, np.int64)
    blk2j = np.empty(n_blocks, np.int64)
    for si, sbl in enumerate(sb_list):
        for j, bb in enumerate(sbl):
            blk2sb[bb] = si
            blk2j[bb] = j

    # --- shard + sort edges per core, per-(sb,q) counts ---
    cores = []
    cnt = np.zeros((N_CORES, NG), np.int64)
    cnt2 = np.zeros((N_CORES, NG, SBW), np.int64)
    for c in range(N_CORES):
        lo = c * nsh
        m = (edge_row >= lo) & (edge_row < min(N, lo + nsh))
        r = edge_row[m] - lo
        ci = edge_col[m]
        v = edge_val[m]
        blk = r >> 7
        q = ci // CH
        sbid = blk2sb[blk]
        jloc = blk2j[blk]
        order = np.lexsort((jloc, q, sbid))
        r, ci, v, q, sbid, jloc = (a[order] for a in (r, ci, v, q, sbid, jloc))
        gid = sbid * Q + q
        cnt[c] = np.bincount(gid, minlength=NG)
        cnt2[c] = np.bincount(gid * SBW + jloc,
                              minlength=NG * SBW).reshape(NG, SBW)
        cores.append((r, ci, v, q, gid, jloc))

    if ALIGN:
        # each (gid, block) segment gets a fixed 128-aligned reservation:
        # no cross-block matmul instances, more gather padding
        wseg = -(-cnt2.max(axis=0) // P) * P                  # [NG, SBW]
        wseg[:, 0] = np.maximum(wseg[:, 0], P)
        seg_base = np.zeros((NG, SBW + 1), np.int64)
        np.cumsum(wseg, axis=1, out=seg_base[:, 1:])
        nidx_g = seg_base[:, -1]
        n_groups = nidx_g // P
    else:
        n_groups = np.maximum(1, -(-cnt.max(axis=0) // P))    # per gid
        nidx_g = n_groups * P
    slot_base = np.zeros(NG + 1, np.int64)
    np.cumsum(nidx_g, out=slot_base[1:])
    total_slots = int(slot_base[-1])
    G_MAX = int(n_groups.max())

    # --- instance list: per gid, which (group, block) pairs exist ---
    if ALIGN:
        Gs = seg_base[:, :-1] >> 7
        Ge = (seg_base[:, 1:] - 1) >> 7
        Ge = np.where(wseg > 0, Ge, Gs - 1)   # empty segment -> no instance
        Ge[:, 0] = np.maximum(Ge[:, 0], Gs[:, 0])
    else:
        s_cgj = np.zeros((N_CORES, NG, SBW + 1), np.int64)
        np.cumsum(cnt2, axis=2, out=s_cgj[:, :, 1:])
        starts = s_cgj[:, :, :-1]
        ends = s_cgj[:, :, 1:]
        has = cnt2 > 0
        gs = np.where(has, starts >> 7, np.iinfo(np.int64).max)
        ge = np.where(has, (ends - 1) >> 7, -1)
        Gs = gs.min(axis=0)            # [NG, SBW]
        Ge = ge.max(axis=0)
        # guarantee every block of every sb has at least one instance
        none = Ge < 0
        Gs[none] = 0
        Ge[none] = 0

    # bank-merged instances: one matmul per (gid, group, PSUM bank) covering
    # the span of blocks the group touches within that bank (wider one-hot S
    # with bank-local dest offsets instead of one matmul per block)
    NBK = (SBW + 3) // 4
    inst_span = {}                 # key -> [jmin, jmax]
    for gidx in range(NG):
        sbid = gidx // Q
        for j in range(len(sb_list[sbid])):
            for g in range(Gs[gidx, j], Ge[gidx, j] + 1):
                key = (gidx * G_MAX + g) * NBK + j // 4
                sp = inst_span.get(key)
                if sp is None:
                    inst_span[key] = [j, j]
                else:
                    sp[0] = min(sp[0], j)
                    sp[1] = max(sp[1], j)
    inst_keys = np.array(sorted(inst_span), np.int64)
    M = len(inst_keys)

    # decode instances; start/stop per PSUM bank (start zeroes a whole 2KB)
    first_of_bank = {}
    last_of_bank = {}
    inst_decode = []
    for m in range(M):
        k = int(inst_keys[m])
        bank = k % NBK
        g = (k // NBK) % G_MAX
        gidx = k // (NBK * G_MAX)
        sbid, q = gidx // Q, gidx % Q
        jmin, jmax = inst_span[(k // NBK) * NBK + bank]
        c0 = (jmin % 4) * P
        w = (jmax - jmin + 1) * P
        inst_decode.append((sbid, q, g, bank, c0, w))
        bk = (sbid, bank)
        if bk not in first_of_bank:
            first_of_bank[bk] = m
        last_of_bank[bk] = m

    # per-sb schedules with sb-local idx offsets and instance ids
    sbs_sched = []
    m_lo = np.zeros(n_sb + 1, np.int64)       # instance-id range per sb
    for m in range(M):
        m_lo[inst_decode[m][0] + 1] = m + 1
    for sbid in range(n_sb):
        nb = len(sb_list[sbid])
        gathers = []
        ioff = 0
        for q in range(Q):
            gidx = sbid * Q + q
            gathers.append((q, int(nidx_g[gidx]), ioff))
            ioff += int(nidx_g[gidx]) // 16
        instances = []
        for m in range(int(m_lo[sbid]), int(m_lo[sbid + 1])):
            s_, q_, g_, bank_, c0_, w_ = inst_decode[m]
            assert s_ == sbid
            st = first_of_bank[(sbid, bank_)] == m
            sp = last_of_bank[(sbid, bank_)] == m
            if st:
                # start=True zeroes the whole 2KB bank: write all 512 columns
                # (one-hot S leaves untouched dests at exactly 0)
                c0_, w_ = 0, 512
            instances.append((q_, g_, bank_, c0_, w_, st, sp,
                              m - int(m_lo[sbid])))
        sbs_sched.append({"nb": nb, "col0": sb_list[sbid][0] * P,
                          "gathers": gathers, "instances": instances,
                          "ic": ioff, "m": len(instances)})

    sched = {"n_blocks": n_blocks, "Q": Q, "total_slots": total_slots,
             "M": M, "xgw": G_MAX * P, "sbs": sbs_sched,
             "ic_max": max(s["ic"] for s in sbs_sched),
             "m_max": max(s["m"] for s in sbs_sched)}

    # --- per-core tensors ---
    iota_np = np.tile(np.arange(512, dtype=np.float32),
                      (P, 1)).astype(np.float16)
    in_maps = []
    for c in range(N_CORES):
        r, ci, v, q, gid, jloc = cores[c]
        if ALIGN:
            c2 = np.zeros(NG * SBW + 1, np.int64)
            np.cumsum(cnt2[c].reshape(-1), out=c2[1:])
            rank = np.arange(len(r), dtype=np.int64) - c2[gid * SBW + jloc]
            slot = slot_base[gid] + seg_base[gid, jloc] + rank
        else:
            gstart = np.zeros(NG + 1, np.int64)
            np.cumsum(cnt[c], out=gstart[1:])
            rank = np.arange(len(r), dtype=np.int64) - gstart[gid]
            slot = slot_base[gid] + rank

        idx16 = np.zeros(total_slots, np.int16)
        idx16[slot] = (ci - q * CH).astype(np.int16)

        NBK = (SBW + 3) // 4
        ke = (gid * G_MAX + ((slot - slot_base[gid]) >> 7)) * NBK + jloc // 4
        me = np.searchsorted(inst_keys, ke)
        assert (inst_keys[me] == ke).all()
        d_all = np.full((P, M), -1.0, np.float32)
        v_all = np.zeros((P, M), np.float32)
        d_all[slot & 127, me] = ((jloc % 4) * P + (r & 127)).astype(np.float32)
        v_all[slot & 127, me] = v

        degv = np.zeros(nsh_pad, np.float32)
        lo = c * nsh
        hi = min(N, lo + nsh)
        degv[:hi - lo] = np.bincount(r, weights=v, minlength=hi - lo
                                     ).astype(np.float32)[:hi - lo]
        cpk = np.concatenate([W_bf, WresI_bf, iota_np], axis=1)
        rpk = np.concatenate(
            [b_bf.ravel(), bres_bf.ravel(), degv.astype(np.float16)]
        ).reshape(1, 256 + nsh_pad)
        im = {
            "x": x_bf, "cpack": np.ascontiguousarray(cpk),
            "rpack": np.ascontiguousarray(rpk),
        }
        for sbid in range(n_sb):
            g0 = sbid * Q
            sl0, sl1 = int(slot_base[g0]), int(slot_base[g0 + Q])
            seg = idx16[sl0:sl1]
            im[f"idx{sbid}"] = np.tile(
                np.ascontiguousarray(seg.reshape(len(seg) // 16, 16).T),
                (8, 1))
            a0, a1 = int(m_lo[sbid]), int(m_lo[sbid + 1])
            im[f"darr{sbid}"] = np.ascontiguousarray(d_all[:, a0:a1])
            im[f"varr{sbid}"] = np.ascontiguousarray(v_all[:, a0:a1])
        in_maps.append(im)
    meta = dict(N=N, nsh=nsh, n_blocks=n_blocks, nsh_pad=nsh_pad, Q=Q)
    return in_maps, meta, sched


def kernel(x, W, b, Wres, bres, edge_val, edge_row, edge_col):
    in_maps, meta, sched = _prep(x, W, b, Wres, bres,
                                 edge_val, edge_row, edge_col)
    nc = _build(np.asarray(x).shape[0], sched)
    res = run_bass_kernel_spmd(nc, in_maps, core_ids=list(range(N_CORES)))
    N, nsh = meta["N"], meta["nsh"]
    out = np.empty((N, D), np.float32)
    for c in range(N_CORES):
        lo = c * nsh
        hi = min(N, lo + nsh)
        out[lo:hi] = res.results[c]["outT"].T[: hi - lo].astype(np.float32)
    return out


# revision 44
# speedup vs baseline: 1.1592x; 1.0982x over previous
"""GCNConv (graph message passing) on 8 Trainium2 NeuronCores — Bass/Tile.

out = a + (a @ Wres + bres),  a = relu(segment_sum(edge_val * (xW+b)[edge_col],
edge_row)),  computed via the identity  agg_lin = (A@x) @ W + deg x b  so the
sparse part runs on raw x, and the residual is fused as  out = a@(Wres+I)+bres.

Sharding: nodes (segment-sum destinations) are partitioned across the 8 cores
(12500 nodes each); x (host-cast to fp16) and the small dense weights are
replicated; each core processes exactly the edges whose destination lands in
its shard (host-side routing).

Per-core device algorithm (fully transposed, features on partitions):
  Phase 1, per superblock of SBW=8 destination blocks (128 dests each):
  for each of the Q=4 source chunks (int16 gather indices limit a chunk to
  <=32767 rows of x) one dma_gather pulls that chunk's edges' source rows —
  packed DENSELY in (block, slot) order, fp16, 256B per row — into an SBUF
  tile xg [128 slots, n_groups*128 feats].  Each 128-slot group g feeds one
  matmul per destination block it touches: psum[b] += xg[:, g].T @ S where
  S[slot, dest] = (iota==d)*v is built on the vector engine from per-slot
  dest-offset/value scalars (d=-1 masks slots of other blocks / padding, so
  block boundaries may fall mid-group without any padding).  Two PSUM banks
  [128 f, 4*128 d] accumulate across chunks, then flush to fp16 agg tiles.
  Gather indices and the per-instance d/v scalars stream per superblock so
  the first gathers are not serialized behind one big metadata load.
  Phase 2 (interleaved per superblock, slabs of 512 dests): psA = W.T @ agg
  + b x deg (rank-1), aT = relu(psA) in fp16; psB = (Wres+I).T @ aT + bres;
  outT[:, slab] = psB, stored transposed [128, 12544] per core; the host
  transposes + concatenates.
"""
import math

import numpy as np

import concourse.tile as tile
from concourse import bacc, mybir
from concourse.bass_utils import run_bass_kernel_spmd

F32 = mybir.dt.float32
F16 = mybir.dt.float16
I16 = mybir.dt.int16
AL = mybir.AluOpType
D = 128
P = 128
N_CORES = 8
CH = 25000        # x chunk rows (int16 gather indices => <= 32767)
SBW = 8           # destination blocks per superblock
NQ = 4            # SWDGE queues
S_POOL_EVERY = 0  # every Nth one-hot S is built on gpsimd instead of DVE
                  # (0 = never) to relieve the DVE sequencer
XG_BUFS = 13      # xg pool depth
META_BUFS = 5     # idx/d/v pool depth
ALIGN = False     # 128-align each (block, chunk) segment (fewer matmuls,
                  # more gather padding) vs dense packing (opposite)


def _build(n_src, sched, repeat=1):
    n_blocks = sched["n_blocks"]
    nsh_pad = n_blocks * P
    Q = sched["Q"]
    XGW = sched["xgw"]          # fixed xg tile width (cols, feat-major)
    IC_MAX = sched["ic_max"]
    M_MAX = sched["m_max"]
    n_sb = len(sched["sbs"])

    nc = bacc.Bacc("TRN2", target_bir_lowering=False, debug=False,
                   num_swdge_queues=NQ)
    x = nc.dram_tensor("x", [n_src, D], F16, kind="ExternalInput")
    cpack = nc.dram_tensor("cpack", [P, 768], F16, kind="ExternalInput")
    rpack = nc.dram_tensor("rpack", [1, 256 + nsh_pad], F16,
                           kind="ExternalInput")
    n_gath = sum(len(s["gathers"]) for s in sched["sbs"])
    gcnt = nc.dram_tensor("gcnt", [1, n_gath], mybir.dt.int32,
                          kind="ExternalInput")
    idx_t = [nc.dram_tensor(f"idx{k}", [P, sched["sbs"][k]["ic"]], I16,
                            kind="ExternalInput") for k in range(n_sb)]
    d_t = [nc.dram_tensor(f"darr{k}", [P, sched["sbs"][k]["m"]], F32,
                          kind="ExternalInput") for k in range(n_sb)]
    v_t = [nc.dram_tensor(f"varr{k}", [P, sched["sbs"][k]["m"]], F32,
                          kind="ExternalInput") for k in range(n_sb)]
    outT = nc.dram_tensor("outT", [D, nsh_pad], F16, kind="ExternalOutput")

    with tile.TileContext(nc) as tc:
        with tc.tile_pool(name="const", bufs=1) as cp:
            cpk = cp.tile([P, 768], F16)
            nc.sync.dma_start(cpk[:], cpack.ap())
            W_sb = cpk[:, 0:D]
            WresI_sb = cpk[:, D:2 * D]
            iota_f = cpk[:, 2 * D:2 * D + 512]
            rpk = cp.tile([1, 256 + nsh_pad], F16)
            nc.sync.dma_start(rpk[:], rpack.ap())
            b_sb = rpk[:1, 0:D]
            bres_sb = rpk[:1, D:2 * D]
            deg_sb = rpk[:1, 256:]
            ones_row = cp.tile([1, 512], F16)
            nc.vector.memset(ones_row[:], 1.0)
            gcnt_sb = cp.tile([1, n_gath], mybir.dt.int32)
            nc.sync.dma_start(gcnt_sb[:], gcnt.ap())
            cnt_reg = nc.gpsimd.alloc_register("gcnt_reg")

            for _rep in range(repeat):
                with (
                    tc.tile_pool(name="meta", bufs=META_BUFS) as mp,
                    tc.tile_pool(name="xg", bufs=XG_BUFS) as xg_pool,
                    tc.tile_pool(name="s", bufs=24) as s_pool,
                    tc.tile_pool(name="agg", bufs=2) as agg_pool,
                    tc.tile_pool(name="ot", bufs=2) as o_pool,
                    tc.tile_pool(name="ps1", bufs=6,
                                 space="PSUM") as ps1,
                    tc.tile_pool(name="psA", bufs=1, space="PSUM") as psA_pool,
                    tc.tile_pool(name="psB", bufs=1, space="PSUM") as psB_pool,
                ):
                    for sbi, sb in enumerate(sched["sbs"]):
                        nb = sb["nb"]
                        col0 = sb["col0"]
                        ic, m_sb = sb["ic"], sb["m"]
                        idx_sb = mp.tile([P, IC_MAX], I16, tag="idx",
                                         name=f"idx{sbi}")
                        nc.sync.dma_start(idx_sb[:, :ic], idx_t[sbi].ap())
                        d_sb = mp.tile([P, M_MAX], F32, tag="d",
                                       name=f"d{sbi}")
                        nc.sync.dma_start(d_sb[:, :m_sb], d_t[sbi].ap())
                        v_sb = mp.tile([P, M_MAX], F32, tag="v",
                                       name=f"v{sbi}")
                        nc.sync.dma_start(v_sb[:, :m_sb], v_t[sbi].ap())
                        xgs = []
                        for q, nidx, ioff, mg in sb["gathers"]:
                            xg = xg_pool.tile([P, XGW], F16, tag="xg",
                                              name=f"xg{sbi}_{q}")
                            gi = sbi * Q + q
                            if mg > 0:
                                nc.gpsimd.memset(
                                    xg[:, nidx - mg * P: nidx], 0.0)
                            nc.gpsimd.reg_load(cnt_reg,
                                               gcnt_sb[:1, gi:gi + 1])
                            nreg = cnt_reg
                            nc.gpsimd.dma_gather(
                                xg[:, :nidx].rearrange("p (g f) -> p g f", f=P),
                                x.ap()[q * CH: min(n_src, (q + 1) * CH), :],
                                idx_sb[:, ioff: ioff + nidx // 16],
                                nidx, nreg, D,
                                single_packet=(nidx <= 1024),
                                queue_num=q % NQ,
                            )
                            xgs.append(xg)
                        psbanks = [ps1.tile([P, 512], F32, tag="ps",
                                            name=f"ps{sbi}_{k}")
                                   for k in range((nb + 3) // 4)]
                        pss = [psbanks[j // 4][:, (j % 4) * P:(j % 4 + 1) * P]
                               for j in range(nb)]
                        squad, qslot = None, 4
                        for ii, (qi, g, bank, c0, w, st, sp, m) in \
                                enumerate(sb["instances"]):
                            eng = (nc.gpsimd if S_POOL_EVERY and
                                   ii % S_POOL_EVERY == S_POOL_EVERY - 1
                                   else nc.vector)
                            if w == P:
                                # pack 4 narrow S's per pool tile: 4x fewer
                                # WAR semaphore waits on the DVE sequencer
                                if qslot == 4:
                                    squad = s_pool.tile([P, 512], F16,
                                                        name="S")
                                    qslot = 0
                                Sap = squad[:, qslot * P:(qslot + 1) * P]
                                qslot += 1
                            else:
                                Sap = s_pool.tile([P, 512], F16,
                                                  name="S")[:, :w]
                            eng.tensor_scalar(
                                Sap, iota_f[:, c0:c0 + w],
                                d_sb[:, m:m + 1], v_sb[:, m:m + 1],
                                op0=AL.is_equal, op1=AL.mult,
                            )
                            nc.tensor.matmul(
                                out=psbanks[bank][:, c0:c0 + w],
                                lhsT=xgs[qi][:, g * P:(g + 1) * P],
                                rhs=Sap,
                                start=st, stop=sp,
                            )
                        agg_sb = agg_pool.tile([P, SBW * P], F16, tag="agg",
                                               name="agg")
                        for k in range((nb + 3) // 4):
                            bw = min(512, nb * P - k * 512)
                            nc.scalar.activation(
                                agg_sb[:, k * 512:k * 512 + bw],
                                psbanks[k][:, :bw],
                                mybir.ActivationFunctionType.Copy)
                        # dense head on this superblock's columns
                        for s0 in range(0, nb * P, 512):
                            w = min(512, nb * P - s0)
                            psA = psA_pool.tile([P, 512], F32, name="psA")
                            nc.tensor.matmul(out=psA[:, :w], lhsT=W_sb,
                                             rhs=agg_sb[:, s0:s0 + w],
                                             start=True, stop=False)
                            nc.tensor.matmul(
                                out=psA[:, :w], lhsT=b_sb,
                                rhs=deg_sb[:, col0 + s0: col0 + s0 + w],
                                start=False, stop=True)
                            a_t = agg_pool.tile([P, 512], F16, tag="at",
                                                name="at")
                            nc.scalar.activation(
                                a_t[:, :w], psA[:, :w],
                                mybir.ActivationFunctionType.Relu)
                            psB = psB_pool.tile([P, 512], F32, name="psB")
                            nc.tensor.matmul(out=psB[:, :w], lhsT=WresI_sb,
                                             rhs=a_t[:, :w],
                                             start=True, stop=False)
                            nc.tensor.matmul(out=psB[:, :w],
                                             lhsT=bres_sb,
                                             rhs=ones_row[:1, :w],
                                             start=False, stop=True)
                            o_t = o_pool.tile([P, 512], F16, name="ot")
                            nc.scalar.activation(
                                o_t[:, :w], psB[:, :w],
                                mybir.ActivationFunctionType.Copy)
                            nc.sync.dma_start(
                                outT.ap()[:, col0 + s0: col0 + s0 + w],
                                o_t[:, :w])

    nc.compile()
    return nc


def _prep(x, W, b, Wres, bres, edge_val, edge_row, edge_col):
    x = np.asarray(x, np.float32)
    n_src = x.shape[0]
    N = n_src
    x_bf = np.ascontiguousarray(x.astype(np.float16))
    W_bf = np.ascontiguousarray(np.asarray(W, np.float32).astype(np.float16))
    WresI = np.asarray(Wres, np.float32) + np.eye(D, dtype=np.float32)
    WresI_bf = np.ascontiguousarray(WresI.astype(np.float16))
    b_bf = np.asarray(b, np.float32).reshape(1, D).astype(np.float16)
    bres_bf = np.asarray(bres, np.float32).reshape(1, D).astype(np.float16)
    edge_row = np.asarray(edge_row).astype(np.int64)
    edge_col = np.asarray(edge_col).astype(np.int64)
    edge_val = np.asarray(edge_val, np.float32)

    Q = math.ceil(n_src / CH)
    nsh = math.ceil(N / N_CORES)
    n_blocks = math.ceil(nsh / P)
    nsh_pad = n_blocks * P
    # taper BOTH ends: small superblocks first (compute starts sooner) and
    # last (short drain after the final gather); full-width in the middle
    steps = []
    rem = n_blocks
    for head in (2, 3):
        if rem > SBW + 6:
            steps.append(head)
            rem -= head
    tail = []
    for t in (1, 2, 3):
        if rem > SBW + 6:
            tail.append(t)
            rem -= t
    while rem > 0:
        if rem > SBW + 2:
            s_ = SBW
        elif rem > 4:
            s_ = rem - 4
        elif rem > 2:
            s_ = 2
        else:
            s_ = rem
        steps.append(s_)
        rem -= s_
    steps.extend(reversed(tail))
    sb_list = []
    s = 0
    for step in steps:
        sb_list.append(list(range(s, s + step)))
        s += step
    assert s == n_blocks
    n_sb = len(sb_list)
    NG = n_sb * Q                      # gather-group count
    blk2sb = np.empty(n_blocks, np.int64)
    blk2j = np.empty(n_blocks, np.int64)
    for si, sbl in enumerate(sb_list):
        for j, bb in enumerate(sbl):
            blk2sb[bb] = si
            blk2j[bb] = j

    # --- shard + sort edges per core, per-(sb,q) counts ---
    cores = []
    cnt = np.zeros((N_CORES, NG), np.int64)
    cnt2 = np.zeros((N_CORES, NG, SBW), np.int64)
    for c in range(N_CORES):
        lo = c * nsh
        m = (edge_row >= lo) & (edge_row < min(N, lo + nsh))
        r = edge_row[m] - lo
        ci = edge_col[m]
        v = edge_val[m]
        blk = r >> 7
        q = ci // CH
        sbid = blk2sb[blk]
        jloc = blk2j[blk]
        order = np.lexsort((jloc, q, sbid))
        r, ci, v, q, sbid, jloc = (a[order] for a in (r, ci, v, q, sbid, jloc))
        gid = sbid * Q + q
        cnt[c] = np.bincount(gid, minlength=NG)
        cnt2[c] = np.bincount(gid * SBW + jloc,
                              minlength=NG * SBW).reshape(NG, SBW)
        cores.append((r, ci, v, q, gid, jloc))

    if ALIGN:
        # each (gid, block) segment gets a fixed 128-aligned reservation:
        # no cross-block matmul instances, more gather padding
        wseg = -(-cnt2.max(axis=0) // P) * P                  # [NG, SBW]
        wseg[:, 0] = np.maximum(wseg[:, 0], P)
        seg_base = np.zeros((NG, SBW + 1), np.int64)
        np.cumsum(wseg, axis=1, out=seg_base[:, 1:])
        nidx_g = seg_base[:, -1]
        n_groups = nidx_g // P
    else:
        n_groups = np.maximum(1, -(-cnt.max(axis=0) // P))    # per gid
        nidx_g = n_groups * P
    slot_base = np.zeros(NG + 1, np.int64)
    np.cumsum(nidx_g, out=slot_base[1:])
    total_slots = int(slot_base[-1])
    G_MAX = int(n_groups.max())

    # --- instance list: per gid, which (group, block) pairs exist ---
    if ALIGN:
        Gs = seg_base[:, :-1] >> 7
        Ge = (seg_base[:, 1:] - 1) >> 7
        Ge = np.where(wseg > 0, Ge, Gs - 1)   # empty segment -> no instance
        Ge[:, 0] = np.maximum(Ge[:, 0], Gs[:, 0])
    else:
        s_cgj = np.zeros((N_CORES, NG, SBW + 1), np.int64)
        np.cumsum(cnt2, axis=2, out=s_cgj[:, :, 1:])
        starts = s_cgj[:, :, :-1]
        ends = s_cgj[:, :, 1:]
        has = cnt2 > 0
        gs = np.where(has, starts >> 7, np.iinfo(np.int64).max)
        ge = np.where(has, (ends - 1) >> 7, -1)
        Gs = gs.min(axis=0)            # [NG, SBW]
        Ge = ge.max(axis=0)
        # guarantee every block of every sb has at least one instance
        none = Ge < 0
        Gs[none] = 0
        Ge[none] = 0

    # bank-merged instances: one matmul per (gid, group, PSUM bank) covering
    # the span of blocks the group touches within that bank (wider one-hot S
    # with bank-local dest offsets instead of one matmul per block)
    NBK = (SBW + 3) // 4
    inst_span = {}                 # key -> [jmin, jmax]
    for gidx in range(NG):
        sbid = gidx // Q
        for j in range(len(sb_list[sbid])):
            for g in range(Gs[gidx, j], Ge[gidx, j] + 1):
                key = (gidx * G_MAX + g) * NBK + j // 4
                sp = inst_span.get(key)
                if sp is None:
                    inst_span[key] = [j, j]
                else:
                    sp[0] = min(sp[0], j)
                    sp[1] = max(sp[1], j)
    inst_keys = np.array(sorted(inst_span), np.int64)
    M = len(inst_keys)

    # decode instances; start/stop per PSUM bank (start zeroes a whole 2KB)
    first_of_bank = {}
    last_of_bank = {}
    inst_decode = []
    for m in range(M):
        k = int(inst_keys[m])
        bank = k % NBK
        g = (k // NBK) % G_MAX
        gidx = k // (NBK * G_MAX)
        sbid, q = gidx // Q, gidx % Q
        jmin, jmax = inst_span[(k // NBK) * NBK + bank]
        c0 = (jmin % 4) * P
        w = (jmax - jmin + 1) * P
        inst_decode.append((sbid, q, g, bank, c0, w))
        bk = (sbid, bank)
        if bk not in first_of_bank:
            first_of_bank[bk] = m
        last_of_bank[bk] = m

    # every gather passes its per-core valid count in a register (real HW
    # skips trailing -1 descriptors); the tail groups that may stay unwritten
    # are memset to zero first, so no stale SBUF is ever read
    cnt_min = np.maximum(cnt.min(axis=0), 1)

    # per-sb schedules with sb-local idx offsets and instance ids
    sbs_sched = []
    m_lo = np.zeros(n_sb + 1, np.int64)       # instance-id range per sb
    for m in range(M):
        m_lo[inst_decode[m][0] + 1] = m + 1
    for sbid in range(n_sb):
        nb = len(sb_list[sbid])
        gathers = []
        ioff = 0
        for q in range(Q):
            gidx = sbid * Q + q
            g0 = int(cnt_min[gidx] - 1) >> 7
            gathers.append((q, int(nidx_g[gidx]), ioff,
                            int(n_groups[gidx] - g0)))
            ioff += int(nidx_g[gidx]) // 16
        instances = []
        for m in range(int(m_lo[sbid]), int(m_lo[sbid + 1])):
            s_, q_, g_, bank_, c0_, w_ = inst_decode[m]
            assert s_ == sbid
            st = first_of_bank[(sbid, bank_)] == m
            sp = last_of_bank[(sbid, bank_)] == m
            if st:
                # start=True zeroes the whole 2KB bank: write all 512 columns
                # (one-hot S leaves untouched dests at exactly 0)
                c0_, w_ = 0, 512
            instances.append((q_, g_, bank_, c0_, w_, st, sp,
                              m - int(m_lo[sbid])))
        sbs_sched.append({"nb": nb, "col0": sb_list[sbid][0] * P,
                          "gathers": gathers, "instances": instances,
                          "ic": ioff, "m": len(instances)})

    sched = {"n_blocks": n_blocks, "Q": Q, "total_slots": total_slots,
             "M": M, "xgw": G_MAX * P, "sbs": sbs_sched,
             "ic_max": max(s["ic"] for s in sbs_sched),
             "m_max": max(s["m"] for s in sbs_sched)}

    # --- per-core tensors ---
    iota_np = np.tile(np.arange(512, dtype=np.float32),
                      (P, 1)).astype(np.float16)
    in_maps = []
    for c in range(N_CORES):
        r, ci, v, q, gid, jloc = cores[c]
        if ALIGN:
            c2 = np.zeros(NG * SBW + 1, np.int64)
            np.cumsum(cnt2[c].reshape(-1), out=c2[1:])
            rank = np.arange(len(r), dtype=np.int64) - c2[gid * SBW + jloc]
            slot = slot_base[gid] + seg_base[gid, jloc] + rank
        else:
            gstart = np.zeros(NG + 1, np.int64)
            np.cumsum(cnt[c], out=gstart[1:])
            rank = np.arange(len(r), dtype=np.int64) - gstart[gid]
            slot = slot_base[gid] + rank

        idx16 = np.full(total_slots, -1, np.int16)
        idx16[slot] = (ci - q * CH).astype(np.int16)
        gvals = np.zeros(NG, np.int32)
        for gidx in range(NG):
            cval = int(cnt[c][gidx])
            if cval == 0:
                idx16[int(slot_base[gidx])] = 0   # >=1 valid index required
                gvals[gidx] = 1
            else:
                gvals[gidx] = cval

        NBK = (SBW + 3) // 4
        ke = (gid * G_MAX + ((slot - slot_base[gid]) >> 7)) * NBK + jloc // 4
        me = np.searchsorted(inst_keys, ke)
        assert (inst_keys[me] == ke).all()
        d_all = np.full((P, M), -1.0, np.float32)
        v_all = np.zeros((P, M), np.float32)
        d_all[slot & 127, me] = ((jloc % 4) * P + (r & 127)).astype(np.float32)
        v_all[slot & 127, me] = v

        degv = np.zeros(nsh_pad, np.float32)
        lo = c * nsh
        hi = min(N, lo + nsh)
        degv[:hi - lo] = np.bincount(r, weights=v, minlength=hi - lo
                                     ).astype(np.float32)[:hi - lo]
        cpk = np.concatenate([W_bf, WresI_bf, iota_np], axis=1)
        rpk = np.concatenate(
            [b_bf.ravel(), bres_bf.ravel(), degv.astype(np.float16)]
        ).reshape(1, 256 + nsh_pad)
        im = {
            "x": x_bf, "cpack": np.ascontiguousarray(cpk),
            "rpack": np.ascontiguousarray(rpk),
            "gcnt": gvals.reshape(1, NG),
        }
        for sbid in range(n_sb):
            g0 = sbid * Q
            sl0, sl1 = int(slot_base[g0]), int(slot_base[g0 + Q])
            seg = idx16[sl0:sl1]
            im[f"idx{sbid}"] = np.tile(
                np.ascontiguousarray(seg.reshape(len(seg) // 16, 16).T),
                (8, 1))
            a0, a1 = int(m_lo[sbid]), int(m_lo[sbid + 1])
            im[f"darr{sbid}"] = np.ascontiguousarray(d_all[:, a0:a1])
            im[f"varr{sbid}"] = np.ascontiguousarray(v_all[:, a0:a1])
        in_maps.append(im)
    meta = dict(N=N, nsh=nsh, n_blocks=n_blocks, nsh_pad=nsh_pad, Q=Q)
    return in_maps, meta, sched


def kernel(x, W, b, Wres, bres, edge_val, edge_row, edge_col):
    in_maps, meta, sched = _prep(x, W, b, Wres, bres,
                                 edge_val, edge_row, edge_col)
    nc = _build(np.asarray(x).shape[0], sched)
    res = run_bass_kernel_spmd(nc, in_maps, core_ids=list(range(N_CORES)))
    N, nsh = meta["N"], meta["nsh"]
    out = np.empty((N, D), np.float32)
    for c in range(N_CORES):
        lo = c * nsh
        hi = min(N, lo + nsh)
        out[lo:hi] = res.results[c]["outT"].T[: hi - lo].astype(np.float32)
    return out
